# revision 18
# baseline (speedup 1.0000x reference)
"""Trainium2 Bass kernel for nn_BiMultiHeadAttention (GLIP-style bi-directional MHA).

Sharding: pure data parallelism — batch B=8, one batch element per NeuronCore,
weights replicated, no collectives.

Per-core math (one batch element; H=8 heads, D=256 head dim):
  q^T_h = (Wq_v^T v^T) * s + bq*s        [D, Hv]   (d on partitions)
  k^T_h = Wk_l^T l^T + bk                [D, Hl]
  lv_h  = l Wval_l + bval_l              [Hl, D]
  vv_h  = v Wval_v + bval_v              [Hv, D]
  S^T_h = k^T_h^T q^T_h                  [Hl, Hv]  -> E1 = exp(S^T + bias_l[j])
  S_h   = q^T_h^T k^T_h                  [Hv, Hl]  -> E2 = exp(S + bias_v[i])
  r_i = sum_j E1[j,i] (ones matmul), c_j = sum_i E2[i,j] (ones matmul)
  F^T_h = lv_h^T-contracted E1           [D, Hv]   (= (E1^T @ lv)^T)
  H_h   = F^T_h^T Wo_v[h]                [Hv, 256]; out_v_acc += H_h / r
  G^T_h = vv_h^T-contracted E2           [D, Hl]
  OL_h  = G^T_h^T Wo_l[h]                [Hl, 768]; out_l_acc += OL_h / c
  final: out_v[i] = mv[i] ? acc+bo_v : mean_lv@Wo_v+bo_v   (masked-query uniform
  softmax fallback), analogously out_l with ml[j].

exp biases are -10000*(1-mask): exp underflows to exactly 0.0 for masked
entries, matching the reference's exp(NEG - max) == 0.0; fully-masked rows are
replaced by the uniform-attention mean via the final select.

Matmul operands use dtype float32r (fp32 stored with reduced-precision rounding
for the PE array) -> 4x the fp32 matmul rate.
"""

import numpy as np
from contextlib import ExitStack

import concourse.bass as bass
import concourse.mybir as mybir
import concourse.tile as tile
from concourse import bacc
from concourse.bass_utils import run_bass_kernel_spmd

F32 = mybir.dt.float32
F32R = mybir.dt.float32r
I32 = mybir.dt.int32
AF = mybir.ActivationFunctionType
ALU = mybir.AluOpType

B, Hv, Hl = 8, 4096, 256
V_DIM, L_DIM, EMBED, HEADS = 256, 768, 2048, 8
D = EMBED // HEADS            # 256
SCALING = D ** -0.5
NEG = -10000.0
N_CORES = 8

IC = Hv // 512                # 8 i-chunks of 512
ICC = Hv // 128               # 32 i-subchunks of 128
CV = V_DIM // 128             # 2 contraction chunks for v-side
CL = L_DIM // 128             # 6 contraction chunks for l-side
DC = D // 128                 # 2 chunks of the head dim
JC = Hl // 128                # 2 j-chunks


def _make_identity(nc, ident):
    from concourse.masks import make_identity
    make_identity(nc, ident)


def build_nc(trips=None):
    nc = bacc.Bacc("TRN2")

    # ---------------- DRAM I/O ----------------
    v_d = nc.dram_tensor("v", [Hv, V_DIM], F32, kind="ExternalInput")
    l_d = nc.dram_tensor("l", [Hl, L_DIM], F32, kind="ExternalInput")
    mv_d = nc.dram_tensor("attention_mask_v", [Hv], I32, kind="ExternalInput")
    ml_d = nc.dram_tensor("attention_mask_l", [Hl], I32, kind="ExternalInput")
    Wq_d = nc.dram_tensor("Wq_v", [V_DIM, EMBED], F32R, kind="ExternalInput")
    bq_d = nc.dram_tensor("bq_v", [EMBED], F32, kind="ExternalInput")
    Wk_d = nc.dram_tensor("Wk_l", [L_DIM, EMBED], F32R, kind="ExternalInput")
    bk_d = nc.dram_tensor("bk_l", [EMBED], F32, kind="ExternalInput")
    Wvv_d = nc.dram_tensor("Wval_v", [V_DIM, EMBED], F32R, kind="ExternalInput")
    bvv_d = nc.dram_tensor("bval_v", [EMBED], F32, kind="ExternalInput")
    Wvl_d = nc.dram_tensor("Wval_l", [L_DIM, EMBED], F32R, kind="ExternalInput")
    bvl_d = nc.dram_tensor("bval_l", [EMBED], F32, kind="ExternalInput")
    Wov_d = nc.dram_tensor("Wo_v", [EMBED, V_DIM], F32R, kind="ExternalInput")
    bov_d = nc.dram_tensor("bo_v", [V_DIM], F32R, kind="ExternalInput")
    Wol_d = nc.dram_tensor("Wo_l", [EMBED, L_DIM], F32R, kind="ExternalInput")
    bol_d = nc.dram_tensor("bo_l", [L_DIM], F32R, kind="ExternalInput")
    out_v_d = nc.dram_tensor("out_v", [Hv, V_DIM], F32, kind="ExternalOutput")
    out_l_d = nc.dram_tensor("out_l", [Hl, L_DIM], F32, kind="ExternalOutput")

    # DRAM staging for cross-partition reshapes ([1,N] rows -> [128, N/128])
    r_stage = nc.dram_tensor("r_stage", [HEADS, IC, 512], F32, kind="Internal")
    c_stage = nc.dram_tensor("c_stage", [HEADS, Hl], F32, kind="Internal")
    uv_stage = nc.dram_tensor("uv_stage", [HEADS, D], F32, kind="Internal")
    ul_stage = nc.dram_tensor("ul_stage", [HEADS, D], F32, kind="Internal")

    # rearranged dram views
    v_r = v_d.rearrange("(ic p) c -> p ic c", p=128)        # [128, 32, 256]
    l_r = l_d.rearrange("(jc p) c -> p jc c", p=128)        # [128, 2, 768]
    out_v_r = out_v_d.rearrange("(ic p) o -> p ic o", p=128)
    out_l_r = out_l_d.rearrange("(jc p) o -> p jc o", p=128)
    Wq_r = Wq_d.rearrange("(cc p) e -> p cc e", p=128)      # [128, 2, 2048]
    Wk_r = Wk_d.rearrange("(cc p) e -> p cc e", p=128)      # [128, 6, 2048]
    Wvv_r = Wvv_d.rearrange("(cc p) e -> p cc e", p=128)
    Wvl_r = Wvl_d.rearrange("(cc p) e -> p cc e", p=128)
    Wov_r = Wov_d.rearrange("(dc p) o -> p dc o", p=128)    # [128, 16, 256]
    Wol_r = Wol_d.rearrange("(dc p) o -> p dc o", p=128)    # [128, 16, 768]

    with tile.TileContext(nc) as tc, ExitStack() as ctx:
        if trips is not None:
            ctx.enter_context(tc.For_i(0, trips, 1))
        pp = ctx.enter_context(tc.tile_pool(name="pp", bufs=8, space="PSUM"))
        cp = ctx.enter_context(tc.tile_pool(name="cp", bufs=1))  # persistent

        # ---------------- persistent tiles ----------------
        ident = cp.tile([128, 128], F32)
        _make_identity(nc, ident)
        ones_f = cp.tile([128, 1], F32)
        nc.vector.memset(ones_f, 1.0)
        ones_r = cp.tile([128, 1], F32R)           # ones column (matmul lhsT)
        nc.vector.tensor_copy(ones_r[:], ones_f[:])
        onesrow_f = cp.tile([1, 128], F32)
        nc.vector.memset(onesrow_f, 1.0)
        onesrow_r = cp.tile([1, 128], F32R)        # ones row (K=1 replicate matmul)
        nc.vector.tensor_copy(onesrow_r[:], onesrow_f[:])

        vT = cp.tile([128, CV, Hv], F32R)          # v^T  [c, i]
        lT = cp.tile([128, CL, Hl], F32R)          # l^T  [c, j]
        out_v_acc = cp.tile([128, ICC, V_DIM], F32)
        out_l_acc = cp.tile([128, JC, L_DIM], F32)

        # masks / exp biases
        mv_i = cp.tile([128, ICC], I32)
        nc.sync.dma_start(mv_i, mv_d.rearrange("(c p) -> p c", p=128))
        mvf = cp.tile([128, ICC], F32)
        nc.vector.tensor_copy(mvf[:], mv_i[:])
        bias_v = cp.tile([128, ICC], F32)
        nc.vector.tensor_scalar(bias_v[:], mvf[:], -NEG, NEG, ALU.mult, ALU.add)
        ml_i = cp.tile([128, JC], I32)
        nc.sync.dma_start(ml_i, ml_d.rearrange("(c p) -> p c", p=128))
        mlf = cp.tile([128, JC], F32)
        nc.vector.tensor_copy(mlf[:], ml_i[:])
        bias_l = cp.tile([128, JC], F32)
        nc.vector.tensor_scalar(bias_l[:], mlf[:], -NEG, NEG, ALU.mult, ALU.add)

        # projection biases, per-partition layouts
        bq_s = cp.tile([128, EMBED // 128], F32)   # bq * SCALING, [p, chunk]
        nc.sync.dma_start(bq_s, bq_d.rearrange("(c p) -> p c", p=128))
        nc.vector.tensor_scalar_mul(bq_s[:], bq_s[:], SCALING)
        bk_s = cp.tile([128, EMBED // 128], F32)
        nc.sync.dma_start(bk_s, bk_d.rearrange("(c p) -> p c", p=128))

        # replicated value biases [128, EMBED] (broadcast along partitions)
        bvv_row = cp.tile([1, EMBED], F32R)
        nc.gpsimd.dma_start(bvv_row, bvv_d[None, :])   # casting dma f32->f32r
        bvl_row = cp.tile([1, EMBED], F32R)
        nc.gpsimd.dma_start(bvl_row, bvl_d[None, :])
        bvv_row_f = cp.tile([1, EMBED], F32)
        nc.sync.dma_start(bvv_row_f, bvv_d[None, :])

        # uniform-fallback accumulators (built during head loop)
        u_pre_v = cp.tile([1, EMBED], F32)         # sum_j lv[j, :] per head slice
        u_v_sb = cp.tile([1, V_DIM], F32R)         # mean_lv @ Wo_v
        u_l_sb = cp.tile([1, L_DIM], F32R)         # mean_vv @ Wo_l

        # ---------------- setup: transposes ----------------
        with tc.tile_pool(name="setup", bufs=1) as sp:
            v_sb = sp.tile([128, ICC, V_DIM], F32)
            nc.sync.dma_start(v_sb, v_r)
            l_sb = sp.tile([128, JC, L_DIM], F32)
            nc.sync.dma_start(l_sb, l_r)
            for icc in range(ICC):
                for cc in range(CV):
                    t_ps = pp.tile([128, 128], F32, tag="mm")
                    nc.tensor.transpose(t_ps[:], v_sb[:, icc, bass.ts(cc, 128)], ident[:])
                    nc.vector.tensor_copy(vT[:, cc, bass.ts(icc, 128)], t_ps[:])
            for jc in range(JC):
                for cc in range(CL):
                    t_ps = pp.tile([128, 128], F32, tag="mm")
                    nc.tensor.transpose(t_ps[:], l_sb[:, jc, bass.ts(cc, 128)], ident[:])
                    nc.vector.tensor_copy(lT[:, cc, bass.ts(jc, 128)], t_ps[:])

        # sum over i of v^T (for mean_vv): free-dim reduce of vT
        sum_vf = cp.tile([128, CV, 1], F32)
        nc.vector.tensor_reduce(sum_vf[:], vT[:], axis=mybir.AxisListType.X, op=ALU.add)
        sum_vr = cp.tile([128, CV, 1], F32R)
        nc.vector.tensor_copy(sum_vr[:], sum_vf[:])

        # ---------------- head-loop pools ----------------
        head_ctx = ctx.enter_context(ExitStack())
        wq_p = head_ctx.enter_context(tc.tile_pool(name="wq", bufs=2))
        wk_p = head_ctx.enter_context(tc.tile_pool(name="wk", bufs=1))
        wvv_p = head_ctx.enter_context(tc.tile_pool(name="wvv", bufs=2))
        wvl_p = head_ctx.enter_context(tc.tile_pool(name="wvl", bufs=1))
        wov_p = head_ctx.enter_context(tc.tile_pool(name="wov", bufs=2))
        wol_p = head_ctx.enter_context(tc.tile_pool(name="wol", bufs=1))
        kt_p = head_ctx.enter_context(tc.tile_pool(name="kt", bufs=2))
        lv_p = head_ctx.enter_context(tc.tile_pool(name="lv", bufs=2))
        gt_p = head_ctx.enter_context(tc.tile_pool(name="gts", bufs=2))
        qt_p = head_ctx.enter_context(tc.tile_pool(name="qt", bufs=2))
        et_p = head_ctx.enter_context(tc.tile_pool(name="et", bufs=2))
        ft_p = head_ctx.enter_context(tc.tile_pool(name="ft", bufs=2))
        e2_p = head_ctx.enter_context(tc.tile_pool(name="e2", bufs=4))
        vv_p = head_ctx.enter_context(tc.tile_pool(name="vv", bufs=4))
        sm_p = head_ctx.enter_context(tc.tile_pool(name="sm", bufs=2))   # small rows/cols

        for h in range(HEADS):
            e0 = h * D  # embed offset of this head

            # ---- weight slices ----
            wq = wq_p.tile([128, CV, D], F32R, tag="wq")
            nc.sync.dma_start(wq, Wq_r[:, :, e0:e0 + D])
            wk = wk_p.tile([128, CL, D], F32R, tag="wk")
            nc.sync.dma_start(wk, Wk_r[:, :, e0:e0 + D])
            wvv = wvv_p.tile([128, CV, D], F32R, tag="wvv")
            nc.sync.dma_start(wvv, Wvv_r[:, :, e0:e0 + D])
            wvl = wvl_p.tile([128, CL, D], F32R, tag="wvl")
            nc.sync.dma_start(wvl, Wvl_r[:, :, e0:e0 + D])
            wov = wov_p.tile([128, DC, V_DIM], F32R, tag="wov")
            nc.sync.dma_start(wov, Wov_r[:, h * DC:(h + 1) * DC, :])
            wol = wol_p.tile([128, DC, L_DIM], F32R, tag="wol")
            nc.sync.dma_start(wol, Wol_r[:, h * DC:(h + 1) * DC, :])

            bvv_rep = kt_p.tile([128, D], F32, tag="bvvrep")
            rep_ps = pp.tile([128, D], F32, tag="mm")
            nc.tensor.matmul(rep_ps[:], onesrow_r[:], bvv_row[0:1, e0:e0 + D],
                             start=True, stop=True)
            nc.scalar.copy(bvv_rep[:], rep_ps[:])
            bvl_rep = kt_p.tile([128, D], F32, tag="bvlrep")
            rep_ps2 = pp.tile([128, D], F32, tag="mm")
            nc.tensor.matmul(rep_ps2[:], onesrow_r[:], bvl_row[0:1, e0:e0 + D],
                             start=True, stop=True)
            nc.scalar.copy(bvl_rep[:], rep_ps2[:])

            # ---- k^T [d, j] and lv [j, d] ----
            kt = kt_p.tile([128, DC, Hl], F32R, tag="kt")
            for dc in range(DC):
                kt_ps = pp.tile([128, Hl], F32, tag="mm")
                for cc in range(CL):
                    nc.tensor.matmul(kt_ps[:], wk[:, cc, bass.ts(dc, 128)],
                                     lT[:, cc, :], start=(cc == 0), stop=(cc == CL - 1))
                nc.scalar.activation(kt[:, dc, :], kt_ps[:], AF.Identity,
                                     bias=bk_s[:, h * DC + dc:h * DC + dc + 1])
            lv = lv_p.tile([128, JC, D], F32R, tag="lv")
            for jc in range(JC):
                lv_ps = pp.tile([128, D], F32, tag="mm")
                for cc in range(CL):
                    nc.tensor.matmul(lv_ps[:], lT[:, cc, bass.ts(jc, 128)],
                                     wvl[:, cc, :], start=(cc == 0), stop=(cc == CL - 1))
                nc.vector.tensor_tensor(lv[:, jc, :], lv_ps[:],
                                        bvl_rep[:], ALU.add)

            # ---- u_pre_v slice: sum_j lv[j, d] ----
            upv_ps = pp.tile([1, D], F32, tag="mm")
            for jc in range(JC):
                nc.tensor.matmul(upv_ps[:], ones_r[:], lv[:, jc, :],
                                 start=(jc == 0), stop=(jc == JC - 1))
            nc.vector.tensor_scalar_mul(u_pre_v[0:1, e0:e0 + D], upv_ps[:], 1.0 / Hl)

            # ---- u_l contribution: mean_vv_h = sum_v @ Wval_v[:,h]/Hv + bval_v ----
            upl_ps = pp.tile([1, D], F32, tag="mm")
            for cc in range(CV):
                nc.tensor.matmul(upl_ps[:], sum_vr[:, cc, :], wvv[:, cc, :],
                                 start=(cc == 0), stop=(cc == CV - 1))
            mean_vv_row = sm_p.tile([1, D], F32, tag="mvvr")
            nc.vector.scalar_tensor_tensor(mean_vv_row[:], upl_ps[:], 1.0 / Hv,
                                           bvv_row_f[0:1, e0:e0 + D], ALU.mult, ALU.add)
            nc.sync.dma_start(ul_stage[h][None, :], mean_vv_row[:])
            mvv_col = sm_p.tile([128, DC, 1], F32R, tag="mvvc")
            nc.gpsimd.dma_start(mvv_col, ul_stage[h].rearrange("(c p) -> p c", p=128)[:, :, None])
            for oh in range(2):
                o0 = oh * (L_DIM // 2)
                ul_ps = pp.tile([1, L_DIM // 2], F32, tag="mm")
                for cc in range(DC):
                    nc.tensor.matmul(ul_ps[:], mvv_col[:, cc, :],
                                     wol[:, cc, o0:o0 + L_DIM // 2],
                                     start=(cc == 0), stop=(cc == DC - 1))
                if h == 0:
                    nc.vector.tensor_copy(u_l_sb[0:1, o0:o0 + L_DIM // 2], ul_ps[:])
                else:
                    nc.vector.tensor_tensor(u_l_sb[0:1, o0:o0 + L_DIM // 2], ul_ps[:],
                                            u_l_sb[0:1, o0:o0 + L_DIM // 2], ALU.add)

            # ---- G^T / c accumulators in SBUF (short psum groups per i-chunk) ----
            gt_acc = gt_p.tile([128, DC, Hl], F32, tag="gtacc")
            c_acc = sm_p.tile([1, Hl], F32, tag="cacc")

            for ic in range(IC):
                i0 = ic * 512
                # q^T chunk [d, 512]
                qt = qt_p.tile([128, DC, 512], F32R, tag="qt")
                for dc in range(DC):
                    qt_ps = pp.tile([128, 512], F32, tag="mm")
                    for cc in range(CV):
                        nc.tensor.matmul(qt_ps[:], wq[:, cc, bass.ts(dc, 128)],
                                         vT[:, cc, i0:i0 + 512],
                                         start=(cc == 0), stop=(cc == CV - 1))
                    nc.vector.tensor_scalar(qt[:, dc, :], qt_ps[:], SCALING,
                                            bq_s[:, h * DC + dc:h * DC + dc + 1],
                                            ALU.mult, ALU.add)
                # S^T chunks + exp1
                et = et_p.tile([128, JC, 512], F32R, tag="et")
                for jc in range(JC):
                    st_ps = pp.tile([128, 512], F32, tag="mm")
                    for dc in range(DC):
                        nc.tensor.matmul(st_ps[:], kt[:, dc, bass.ts(jc, 128)],
                                         qt[:, dc, :], start=(dc == 0), stop=(dc == DC - 1))
                    nc.scalar.activation(et[:, jc, :], st_ps[:], AF.Exp,
                                         bias=bias_l[:, jc:jc + 1])
                # r = sum_j E1[j, i]
                r_ps = pp.tile([1, 512], F32, tag="mm")
                for jc in range(JC):
                    nc.tensor.matmul(r_ps[:], ones_r[:], et[:, jc, :],
                                     start=(jc == 0), stop=(jc == JC - 1))
                r_row = sm_p.tile([1, 512], F32, tag="rrow")
                nc.vector.tensor_copy(r_row[:], r_ps[:])
                nc.sync.dma_start(r_stage[h, ic][None, :], r_row[:])
                r_rec = sm_p.tile([128, 4], F32, tag="rrec")
                nc.sync.dma_start(r_rec, r_stage[h, ic].rearrange("(q p) -> p q", p=128))
                nc.vector.reciprocal(r_rec[:], r_rec[:])

                # S chunks [i128, j] + exp2 + vv
                e2s, vvs = [], []
                for q in range(4):
                    icc = ic * 4 + q
                    s_ps = pp.tile([128, Hl], F32, tag="mm")
                    for dc in range(DC):
                        nc.tensor.matmul(s_ps[:], qt[:, dc, bass.ts(q, 128)],
                                         kt[:, dc, :], start=(dc == 0), stop=(dc == DC - 1))
                    e2 = e2_p.tile([128, Hl], F32R, tag="e2")
                    nc.scalar.activation(e2[:], s_ps[:], AF.Exp,
                                         bias=bias_v[:, icc:icc + 1])
                    e2s.append(e2)
                    vv_ps = pp.tile([128, D], F32, tag="mm")
                    for cc in range(CV):
                        nc.tensor.matmul(vv_ps[:], vT[:, cc, bass.ts(icc, 128)],
                                         wvv[:, cc, :], start=(cc == 0), stop=(cc == CV - 1))
                    vv = vv_p.tile([128, D], F32R, tag="vv")
                    nc.vector.tensor_tensor(vv[:], vv_ps[:], bvv_rep[:], ALU.add)
                    vvs.append(vv)
                # G^T partial: contiguous 4-accumulation groups, then SBUF add
                for dc in range(DC):
                    gt_part = pp.tile([128, Hl], F32, tag="mm")
                    for q in range(4):
                        nc.tensor.matmul(gt_part[:], vvs[q][:, bass.ts(dc, 128)],
                                         e2s[q][:], start=(q == 0), stop=(q == 3))
                    if ic == 0:
                        nc.vector.tensor_copy(gt_acc[:, dc, :], gt_part[:])
                    else:
                        nc.vector.tensor_tensor(gt_acc[:, dc, :], gt_part[:],
                                                gt_acc[:, dc, :], ALU.add)
                c_part = pp.tile([1, Hl], F32, tag="mm")
                for q in range(4):
                    nc.tensor.matmul(c_part[:], ones_r[:], e2s[q][:],
                                     start=(q == 0), stop=(q == 3))
                if ic == 0:
                    nc.vector.tensor_copy(c_acc[:], c_part[:])
                else:
                    nc.vector.tensor_tensor(c_acc[:], c_part[:], c_acc[:], ALU.add)

                # F^T [d, 512]
                ft = ft_p.tile([128, DC, 512], F32R, tag="ft")
                for dc in range(DC):
                    ft_ps = pp.tile([128, 512], F32, tag="mm")
                    for jc in range(JC):
                        nc.tensor.matmul(ft_ps[:], lv[:, jc, bass.ts(dc, 128)],
                                         et[:, jc, :], start=(jc == 0), stop=(jc == JC - 1))
                    nc.scalar.copy(ft[:, dc, :], ft_ps[:])
                # H chunks + normalized accumulation into out_v_acc
                for q in range(4):
                    icc = ic * 4 + q
                    h_ps = pp.tile([128, V_DIM], F32, tag="mm")
                    for dc in range(DC):
                        nc.tensor.matmul(h_ps[:], ft[:, dc, bass.ts(q, 128)],
                                         wov[:, dc, :], start=(dc == 0), stop=(dc == DC - 1))
                    if h == 0:
                        nc.vector.tensor_scalar_mul(out_v_acc[:, icc, :], h_ps[:],
                                                    r_rec[:, q:q + 1])
                    else:
                        nc.vector.scalar_tensor_tensor(out_v_acc[:, icc, :], h_ps[:],
                                                       r_rec[:, q:q + 1],
                                                       out_v_acc[:, icc, :],
                                                       ALU.mult, ALU.add)

            # ---- per-head tail: G^T -> out_l, c recip ----
            nc.sync.dma_start(c_stage[h][None, :], c_acc[:])
            c_rec = sm_p.tile([128, JC], F32, tag="crec")
            nc.sync.dma_start(c_rec, c_stage[h].rearrange("(jc p) -> p jc", p=128))
            nc.vector.reciprocal(c_rec[:], c_rec[:])

            gt = gt_p.tile([128, DC, Hl], F32R, tag="gtsb")
            nc.vector.tensor_copy(gt[:], gt_acc[:])
            for jc in range(JC):
                for oh in range(2):
                    o0 = oh * (L_DIM // 2)
                    ol_ps = pp.tile([128, L_DIM // 2], F32, tag="mm")
                    for dc in range(DC):
                        nc.tensor.matmul(ol_ps[:], gt[:, dc, bass.ts(jc, 128)],
                                         wol[:, dc, o0:o0 + L_DIM // 2],
                                         start=(dc == 0), stop=(dc == DC - 1))
                    if h == 0:
                        nc.vector.tensor_scalar_mul(out_l_acc[:, jc, o0:o0 + L_DIM // 2],
                                                    ol_ps[:], c_rec[:, jc:jc + 1])
                    else:
                        nc.vector.scalar_tensor_tensor(
                            out_l_acc[:, jc, o0:o0 + L_DIM // 2], ol_ps[:],
                            c_rec[:, jc:jc + 1], out_l_acc[:, jc, o0:o0 + L_DIM // 2],
                            ALU.mult, ALU.add)

            # ---- u_v contribution: mean_lv_h @ Wo_v[h] ----
            mlv_row = sm_p.tile([1, D], F32, tag="mlvr")
            nc.vector.tensor_copy(mlv_row[:], u_pre_v[0:1, e0:e0 + D])
            nc.sync.dma_start(uv_stage[h][None, :], mlv_row[:])
            mlv_col = sm_p.tile([128, DC, 1], F32R, tag="mlvc")
            nc.gpsimd.dma_start(mlv_col, uv_stage[h].rearrange("(c p) -> p c", p=128)[:, :, None])
            uv_ps = pp.tile([1, V_DIM], F32, tag="mm")
            for cc in range(DC):
                nc.tensor.matmul(uv_ps[:], mlv_col[:, cc, :], wov[:, cc, :],
                                 start=(cc == 0), stop=(cc == DC - 1))
            if h == 0:
                nc.vector.tensor_copy(u_v_sb[:], uv_ps[:])
            else:
                nc.vector.tensor_tensor(u_v_sb[:], uv_ps[:], u_v_sb[:], ALU.add)

        # ---------------- final: replicate fallbacks, select, write out ----------------
        head_ctx.close()
        with tc.tile_pool(name="fin", bufs=3) as fp:
            bov_row = fp.tile([1, V_DIM], F32R, tag="brow")
            nc.sync.dma_start(bov_row, bov_d[None, :])
            bol_row = fp.tile([1, L_DIM], F32R, tag="brow2")
            nc.sync.dma_start(bol_row, bol_d[None, :])

            # u_v_rep and UB_v_rep = (u_v + bo_v) replicated
            uvr_ps = pp.tile([128, V_DIM], F32, tag="mm")
            nc.tensor.matmul(uvr_ps[:], onesrow_r[:], u_v_sb[:], start=True, stop=True)
            u_v_rep = fp.tile([128, V_DIM], F32, tag="uvrep")
            nc.vector.tensor_copy(u_v_rep[:], uvr_ps[:])
            ubv_ps = pp.tile([128, V_DIM], F32, tag="mm")
            nc.tensor.matmul(ubv_ps[:], onesrow_r[:], u_v_sb[:], start=True, stop=False,
                             skip_group_check=True)
            nc.tensor.matmul(ubv_ps[:], onesrow_r[:], bov_row[:], start=False, stop=True,
                             skip_group_check=True)
            ub_v_rep = fp.tile([128, V_DIM], F32, tag="ubvrep")
            nc.vector.tensor_copy(ub_v_rep[:], ubv_ps[:])

            u_l_rep = fp.tile([128, L_DIM], F32, tag="ulrep")
            ub_l_rep = fp.tile([128, L_DIM], F32, tag="ublrep")
            for oh in range(2):
                o0 = oh * (L_DIM // 2)
                osl = slice(o0, o0 + L_DIM // 2)
                ulr_ps = pp.tile([128, L_DIM // 2], F32, tag="mm")
                nc.tensor.matmul(ulr_ps[:], onesrow_r[:], u_l_sb[0:1, osl],
                                 start=True, stop=True)
                nc.vector.tensor_copy(u_l_rep[:, osl], ulr_ps[:])
                ubl_ps = pp.tile([128, L_DIM // 2], F32, tag="mm")
                nc.tensor.matmul(ubl_ps[:], onesrow_r[:], u_l_sb[0:1, osl],
                                 start=True, stop=False, skip_group_check=True)
                nc.tensor.matmul(ubl_ps[:], onesrow_r[:], bol_row[0:1, osl],
                                 start=False, stop=True, skip_group_check=True)
                nc.vector.tensor_copy(ub_l_rep[:, osl], ubl_ps[:])

            # out_v[i,:] = mv[i] ? acc + bo : u_v + bo  ==  (acc - u_v)*mv + UB_v
            for icc in range(ICC):
                t1 = fp.tile([128, V_DIM], F32, tag="t1")
                nc.vector.tensor_tensor(t1[:], out_v_acc[:, icc, :], u_v_rep[:],
                                        ALU.subtract)
                sel = fp.tile([128, V_DIM], F32, tag="sel")
                nc.vector.scalar_tensor_tensor(sel[:], t1[:], mvf[:, icc:icc + 1],
                                               ub_v_rep[:], ALU.mult, ALU.add)
                nc.sync.dma_start(out_v_r[:, icc, :], sel[:])
            for jc in range(JC):
                t1 = fp.tile([128, L_DIM], F32, tag="t1l")
                nc.vector.tensor_tensor(t1[:], out_l_acc[:, jc, :], u_l_rep[:],
                                        ALU.subtract)
                sel = fp.tile([128, L_DIM], F32, tag="sell")
                nc.vector.scalar_tensor_tensor(sel[:], t1[:], mlf[:, jc:jc + 1],
                                               ub_l_rep[:], ALU.mult, ALU.add)
                nc.sync.dma_start(out_l_r[:, jc, :], sel[:])

    nc.compile()
    return nc


_NC_CACHE = None


def _get_nc():
    global _NC_CACHE
    if _NC_CACHE is None:
        _NC_CACHE = build_nc()
    return _NC_CACHE


def _run(inputs, trace=False):
    nc = _get_nc()
    names = ["v", "l", "attention_mask_v", "attention_mask_l",
             "Wq_v", "bq_v", "Wk_l", "bk_l", "Wval_v", "bval_v",
             "Wval_l", "bval_l", "Wo_v", "bo_v", "Wo_l", "bo_l"]
    per_batch = {"v", "l", "attention_mask_v", "attention_mask_l"}
    in_maps = []
    for b in range(N_CORES):
        m = {}
        for n in names:
            a = np.ascontiguousarray(np.asarray(inputs[n]))
            m[n] = a[b] if n in per_batch else a
        in_maps.append(m)
    res = run_bass_kernel_spmd(nc, in_maps, core_ids=list(range(N_CORES)),
                               trace=trace)
    out_v = np.stack([res.results[b]["out_v"] for b in range(N_CORES)])
    out_l = np.stack([res.results[b]["out_l"] for b in range(N_CORES)])
    return (out_v, out_l), res


def kernel(**inputs):
    (out_v, out_l), _ = _run(inputs, trace=False)
    return out_v, out_l


if __name__ == "__main__":
    import sys
    trips = int(sys.argv[1]) if len(sys.argv) > 1 else None
    nc = build_nc(trips)
    print("build + compile OK", trips)
